# revision 20
# baseline (speedup 1.0000x reference)
"""Trainium2 Bass kernel for nn_EstimatorCV (segment_reduce, 8 NeuronCores).

Data-parallel over N: each of the 8 cores reads a 16384-row shard of
`features`, computes per-class partial sums (sum / sumsq / count) via
one-hot matmuls on the TensorEngine, the [C, 2*(A+1)] partials are
all-reduced across the 8 cores, and every core redundantly computes the
tiny EMA combine and writes the [C,A] outputs.

Host-side prep per core:
  - append a ones column to features ([16384, 257]) so the count falls
    out of the same matmuls (column 256 of the partials),
  - reorder rows so each 1 MiB DMA lands contiguous per SBUF partition,
  - transpose labels to [128 partitions, 128 tiles] float32.

The device work runs in a subprocess with a clean JAX environment so the
caller's JAX state (e.g. JAX_PLATFORMS=cpu) cannot break the PJRT path.
"""

import os
import subprocess
import sys
import tempfile

import numpy as np

N_CORES = 8
N, A, C = 131072, 256, 19
RPC = N // N_CORES  # rows per core = 16384
P = 128  # SBUF partitions / rows per matmul tile
AP1 = A + 2  # 258: features + two ones columns (even, for fp32r matmul)
CP = 20  # classes padded to even (fp32r-producing op constraint)
SUB = 16  # row-tiles per DMA group
GROUPS = RPC // (P * SUB)  # 8 groups of ~2.1MiB per core
GCOLS = SUB * AP1  # 4128 f32 per partition per group
N_TILES = GROUPS * SUB  # 128 row-tiles per core


def _prep_core_inputs(features, labels, core):
    """Build the in_map for one core from the full inputs."""
    sh = features[core * RPC : (core + 1) * RPC]
    f = np.empty((RPC, AP1), np.float32)
    f[:, :A] = sh
    f[:, A:] = 1.0
    # [RPC, AP1] -> [GROUPS, P, SUB*AP1] with rows interleaved so that
    # group g, partition p, subtile j holds row g*P*SUB + j*P + p.
    f = (
        f.reshape(GROUPS, SUB, P, AP1)
        .transpose(0, 2, 1, 3)
        .reshape(GROUPS, P, GCOLS)
    )
    lt = (
        labels[core * RPC : (core + 1) * RPC]
        .reshape(N_TILES, P)
        .T.astype(np.float32)
    )
    return {
        "features": np.ascontiguousarray(f),
        "labels": np.ascontiguousarray(lt),
    }


def _build():
    """Construct the Bass/Tile program (one SPMD NeuronCore view)."""
    from contextlib import ExitStack

    import concourse.mybir as mybir
    import concourse.tile as tile
    from concourse import bacc

    f32 = mybir.dt.float32
    nc = bacc.Bacc(trn_type="TRN2", num_devices=N_CORES)

    feats = nc.dram_tensor("features", [GROUPS, P, GCOLS], f32, kind="ExternalInput")
    labels = nc.dram_tensor("labels", [P, N_TILES], f32, kind="ExternalInput")
    part_out = nc.dram_tensor("part_out", [CP, 2 * AP1], f32, kind="ExternalOutput")

    eq = mybir.AluOpType.is_equal
    f32r = mybir.dt.float32r

    with tile.TileContext(nc) as tc, ExitStack() as ctx:
        const_pool = ctx.enter_context(tc.tile_pool(name="const", bufs=1))
        feat_pool = ctx.enter_context(tc.tile_pool(name="feat", bufs=3))
        featr_pool = ctx.enter_context(tc.tile_pool(name="featr", bufs=2))
        sq_pool = ctx.enter_context(tc.tile_pool(name="sq", bufs=2))
        psum_pool = ctx.enter_context(tc.tile_pool(name="psum", bufs=1, space="PSUM"))
        tail_pool = ctx.enter_context(tc.tile_pool(name="tail", bufs=1))

        # --- constants / small loads ---
        # labels go via the ACT HWDGE queue so the SP queue starts with
        # the first feature group immediately.
        labels_t = const_pool.tile([P, N_TILES], f32)
        nc.scalar.dma_start(labels_t[:], labels.ap())

        # one-hot for ALL 128 row-tiles in one iota + one is_equal:
        # oh_all[p, t*CP + c] = (labels_t[p, t] == c)
        iota_i = const_pool.tile([P, N_TILES * CP], mybir.dt.int32)
        nc.gpsimd.iota(
            iota_i[:].rearrange("p (t c) -> p t c", c=CP),
            pattern=[[0, N_TILES], [1, CP]],
            base=0,
            channel_multiplier=0,
        )
        iota_f = const_pool.tile([P, N_TILES * CP], f32)
        nc.vector.tensor_copy(iota_f[:], iota_i[:])
        oh_all = const_pool.tile([P, N_TILES * CP], f32r)
        nc.vector.tensor_tensor(
            out=oh_all[:].rearrange("p (t c) -> p t c", c=CP),
            in0=iota_f[:].rearrange("p (t c) -> p t c", c=CP),
            in1=labels_t[:].unsqueeze(2).to_broadcast([P, N_TILES, CP]),
            op=eq,
        )

        # --- main loop: per-class partial sums into PSUM ---
        psum_s = psum_pool.tile([CP, AP1], f32)  # [sum(x) | count]
        psum_q = psum_pool.tile([CP, AP1], f32)  # [sum(x^2) | count]

        for g in range(GROUPS):
            feat = feat_pool.tile([P, GCOLS], f32)
            nc.sync.dma_start(feat[:], feats.ap()[g])
            sq = sq_pool.tile([P, GCOLS], f32r)
            nc.scalar.square(sq[:], feat[:])
            featr = featr_pool.tile([P, GCOLS], f32r)
            nc.vector.tensor_copy(featr[:], feat[:])
            for j in range(SUB):
                it = g * SUB + j
                oh = oh_all[:, it * CP : (it + 1) * CP]
                first = it == 0
                last = it == N_TILES - 1
                nc.tensor.matmul(
                    psum_s[:],
                    lhsT=oh,
                    rhs=featr[:, j * AP1 : (j + 1) * AP1],
                    start=first,
                    stop=last,
                )
                nc.tensor.matmul(
                    psum_q[:],
                    lhsT=oh,
                    rhs=sq[:, j * AP1 : (j + 1) * AP1],
                    start=first,
                    stop=last,
                )

        # --- write the per-core partials; reduce + EMA happen on host ---
        part = tail_pool.tile([CP, 2 * AP1], f32)
        nc.vector.tensor_copy(part[:, 0:AP1], psum_s[:])
        nc.vector.tensor_copy(part[:, AP1 : 2 * AP1], psum_q[:])
        nc.sync.dma_start(part_out.ap(), part[:])

    nc.compile()
    return nc


def _host_combine(parts, cov, ave_in, amt):
    """8-way partial reduce + EMA combine (tiny [C,A] math, on host)."""
    red = np.sum(np.asarray(parts, dtype=np.float64), axis=0)[:C]
    s = red[:, 0:A].astype(np.float32)
    cnt = red[:, A].astype(np.float32)
    sq = red[:, AP1 : AP1 + A].astype(np.float32)
    cnt_c = np.maximum(cnt, 1.0)[:, None]
    ave = s / cnt_c
    var = (sq - 2.0 * ave * s + cnt[:, None] * ave * ave) / cnt_c
    denom = cnt + amt
    w = np.where(denom > 0, cnt / np.where(denom > 0, denom, 1.0), 0.0)[:, None]
    additional = w * (1.0 - w) * (ave_in - ave) ** 2
    cov_new = (cov * (1.0 - w) + var * w + additional).astype(np.float32)
    ave_new = (ave_in * (1.0 - w) + ave * w).astype(np.float32)
    amount_new = (amt + cnt).astype(np.float32)
    return cov_new, ave_new, amount_new


def _run_on_device(features, labels, cov, ave, amt, trace=False, tmpdir=None):
    """Shard inputs, compile + execute on 8 NeuronCores, return outputs.

    Must run in an interpreter whose JAX sees the axon NeuronCore devices.
    Returns (cov_new, ave_new, amount_new, exec_time_ns_or_None).
    """
    from concourse.bass_utils import run_bass_kernel_spmd

    nc = _build()
    in_maps = [_prep_core_inputs(features, labels, c) for c in range(N_CORES)]
    res = run_bass_kernel_spmd(
        nc,
        in_maps,
        list(range(N_CORES)),
        trace=trace,
        tmpdir=tmpdir,
    )
    parts = [res.results[c]["part_out"] for c in range(N_CORES)]
    cov_new, ave_new, amount_new = _host_combine(parts, cov, ave, amt)
    return cov_new, ave_new, amount_new, res.exec_time_ns


def _worker_main(argv):
    in_path, out_path = argv[0], argv[1]
    trace = "--trace" in argv
    dat = np.load(in_path)
    cov, ave, amt, exec_ns = _run_on_device(
        dat["features"],
        dat["labels"],
        dat["cov"],
        dat["ave"],
        dat["amt"],
        trace=trace,
        tmpdir=(argv[argv.index("--trace") + 1] if trace else None),
    )
    np.savez(
        out_path,
        cov=cov,
        ave=ave,
        amt=amt,
        exec_ns=np.int64(exec_ns if exec_ns is not None else -1),
    )


def kernel(features, labels, CoVariance, Ave, Amount):
    features = np.ascontiguousarray(np.asarray(features), dtype=np.float32)
    labels = np.ascontiguousarray(np.asarray(labels), dtype=np.int64)
    cov = np.ascontiguousarray(np.asarray(CoVariance), dtype=np.float32)
    ave = np.ascontiguousarray(np.asarray(Ave), dtype=np.float32)
    amt = np.ascontiguousarray(np.asarray(Amount), dtype=np.float32)

    with tempfile.TemporaryDirectory() as td:
        in_path = os.path.join(td, "in.npz")
        out_path = os.path.join(td, "out.npz")
        np.savez(in_path, features=features, labels=labels, cov=cov, ave=ave, amt=amt)
        env = dict(os.environ)
        env.pop("JAX_PLATFORMS", None)  # worker needs the axon NC devices
        subprocess.run(
            [sys.executable, os.path.abspath(__file__), "--_worker", in_path, out_path],
            check=True,
            env=env,
        )
        out = np.load(out_path)
        return out["cov"], out["ave"], out["amt"]


if __name__ == "__main__":
    if len(sys.argv) > 1 and sys.argv[1] == "--_worker":
        _worker_main(sys.argv[2:])
    else:
        sys.exit("usage: kernel.py --_worker IN OUT [--trace DIR]")


# revision 21
# speedup vs baseline: 1.1591x; 1.1591x over previous
"""Trainium2 Bass kernel for nn_EstimatorCV (segment_reduce, 8 NeuronCores).

Data-parallel over N: each of the 8 cores reads a 16384-row shard of
`features` and computes per-class partial sums (sum and sum-of-squares)
with one-hot fp32r matmuls on the TensorEngine, accumulated in PSUM.
The per-core [20, 512] partials are summed on the host together with a
`bincount` of the labels (the per-class counts), followed by the tiny
[C, A] EMA combine — microseconds of host work, which avoids the
~40 us on-device AllReduce floor for a 39 KB collective.

Device layout per core (16384 rows = 8 groups x 16 row-tiles x 128):
  - features are host-reordered so each group is one contiguous-per-
    partition ~2 MiB DMA,
  - ACT squares each group, DVE rounds features to fp32r; both land
    interleaved in a combo tile so each row-tile needs ONE [128,20] x
    [128,512] matmul (psum[c] += onehot^T @ [x | x^2]),
  - one-hots for all 128 row-tiles are built by a single iota +
    is_equal pair.

The device work runs in a subprocess with a clean JAX environment so
the caller's JAX state (e.g. JAX_PLATFORMS=cpu) cannot break the PJRT
path.
"""

import os
import subprocess
import sys
import tempfile

import numpy as np

N_CORES = 8
N, A, C = 131072, 256, 19
RPC = N // N_CORES  # rows per core = 16384
P = 128  # SBUF partitions / rows per matmul tile
CP = 20  # classes padded to even (fp32r constraint)
SUB = 16  # row-tiles per DMA group
CHUNK = 4  # row-tiles per square/round chunk
GROUPS = RPC // (P * SUB)  # 8 groups of 2 MiB per core
GCOLS = SUB * A  # 4096 f32 per partition per group
N_TILES = GROUPS * SUB  # 128 row-tiles per core
W2 = 2 * A  # 512: [feat | feat^2] matmul width


def _prep_core_inputs(features, labels, core):
    """Build the in_map for one core from the full inputs."""
    sh = features[core * RPC : (core + 1) * RPC]
    # [RPC, A] -> [GROUPS, P, SUB*A] with rows interleaved so that
    # group g, partition p, subtile j holds row g*P*SUB + j*P + p.
    f = (
        np.ascontiguousarray(sh, dtype=np.float32)
        .reshape(GROUPS, SUB, P, A)
        .transpose(0, 2, 1, 3)
        .reshape(GROUPS, P, GCOLS)
    )
    lt = (
        labels[core * RPC : (core + 1) * RPC]
        .reshape(N_TILES, P)
        .T.astype(np.float32)
    )
    return {
        "features": np.ascontiguousarray(f),
        "labels": np.ascontiguousarray(lt),
    }


def _build():
    """Construct the Bass/Tile program (one SPMD NeuronCore view)."""
    from contextlib import ExitStack

    import concourse.mybir as mybir
    import concourse.tile as tile
    from concourse import bacc

    f32 = mybir.dt.float32
    f32r = mybir.dt.float32r
    nc = bacc.Bacc(trn_type="TRN2", num_devices=N_CORES)

    feats = nc.dram_tensor("features", [GROUPS, P, GCOLS], f32, kind="ExternalInput")
    labels = nc.dram_tensor("labels", [P, N_TILES], f32, kind="ExternalInput")
    part_out = nc.dram_tensor("part_out", [CP, W2], f32, kind="ExternalOutput")

    eq = mybir.AluOpType.is_equal

    with tile.TileContext(nc) as tc, ExitStack() as ctx:
        const_pool = ctx.enter_context(tc.tile_pool(name="const", bufs=1))
        feat_pool = ctx.enter_context(tc.tile_pool(name="feat", bufs=3))
        combo_pool = ctx.enter_context(tc.tile_pool(name="combo", bufs=2))
        psum_pool = ctx.enter_context(tc.tile_pool(name="psum", bufs=1, space="PSUM"))
        tail_pool = ctx.enter_context(tc.tile_pool(name="tail", bufs=1))

        # --- constants / small loads ---
        # labels go via the ACT HWDGE queue so the SP queue starts with
        # the first feature group immediately.
        labels_t = const_pool.tile([P, N_TILES], f32)
        nc.scalar.dma_start(labels_t[:], labels.ap())

        # one-hot for ALL 128 row-tiles in one iota + one is_equal:
        # oh_all[p, t*CP + c] = (labels_t[p, t] == c)
        iota_i = const_pool.tile([P, N_TILES * CP], mybir.dt.int32)
        nc.gpsimd.iota(
            iota_i[:].rearrange("p (t c) -> p t c", c=CP),
            pattern=[[0, N_TILES], [1, CP]],
            base=0,
            channel_multiplier=0,
        )
        iota_f = const_pool.tile([P, N_TILES * CP], f32)
        nc.vector.tensor_copy(iota_f[:], iota_i[:])
        oh_all = const_pool.tile([P, N_TILES * CP], f32r)
        nc.vector.tensor_tensor(
            out=oh_all[:].rearrange("p (t c) -> p t c", c=CP),
            in0=iota_f[:].rearrange("p (t c) -> p t c", c=CP),
            in1=labels_t[:].unsqueeze(2).to_broadcast([P, N_TILES, CP]),
            op=eq,
        )

        # --- main loop: psum[c, :] += onehot_j^T @ [x_j | x_j^2] ---
        psum = psum_pool.tile([CP, W2], f32)

        for g in range(GROUPS):
            feat = feat_pool.tile([P, GCOLS], f32)
            nc.sync.dma_start(feat[:], feats.ap()[g])
            combo = combo_pool.tile([P, SUB * W2], f32r)
            combo_v = combo[:].rearrange("p (j x) -> p j x", x=W2)
            feat_v = feat[:].rearrange("p (j a) -> p j a", a=A)
            for c in range(SUB // CHUNK):
                cs = slice(c * CHUNK, (c + 1) * CHUNK)
                # fp32r-rounded features into the left half of each
                # row-tile's combo slot, squares into the right half.
                nc.vector.tensor_copy(combo_v[:, cs, 0:A], feat_v[:, cs, :])
                nc.scalar.square(combo_v[:, cs, A:W2], feat_v[:, cs, :])
                for j in range(c * CHUNK, (c + 1) * CHUNK):
                    it = g * SUB + j
                    nc.tensor.matmul(
                        psum[:],
                        lhsT=oh_all[:, it * CP : (it + 1) * CP],
                        rhs=combo_v[:, j, :],
                        start=(it == 0),
                        stop=(it == N_TILES - 1),
                    )

        # --- write the per-core partials; reduce + EMA happen on host ---
        part = tail_pool.tile([CP, W2], f32)
        nc.vector.tensor_copy(part[:], psum[:])
        nc.sync.dma_start(part_out.ap(), part[:])

    nc.compile()
    return nc


def _host_combine(parts, labels, cov, ave_in, amt):
    """8-way partial reduce + count + EMA combine (tiny, on host)."""
    red = np.sum(np.asarray(parts, dtype=np.float64), axis=0)[:C]
    s = red[:, 0:A].astype(np.float32)
    sq = red[:, A:W2].astype(np.float32)
    cnt = np.bincount(np.asarray(labels).ravel(), minlength=C).astype(np.float32)[:C]
    cnt_c = np.maximum(cnt, 1.0)[:, None]
    ave = s / cnt_c
    var = (sq - 2.0 * ave * s + cnt[:, None] * ave * ave) / cnt_c
    denom = cnt + amt
    w = np.where(denom > 0, cnt / np.where(denom > 0, denom, 1.0), 0.0)[:, None]
    additional = w * (1.0 - w) * (ave_in - ave) ** 2
    cov_new = (cov * (1.0 - w) + var * w + additional).astype(np.float32)
    ave_new = (ave_in * (1.0 - w) + ave * w).astype(np.float32)
    amount_new = (amt + cnt).astype(np.float32)
    return cov_new, ave_new, amount_new


def _run_on_device(features, labels, cov, ave, amt, trace=False, tmpdir=None):
    """Shard inputs, compile + execute on 8 NeuronCores, return outputs.

    Must run in an interpreter whose JAX sees the axon NeuronCore devices.
    Returns (cov_new, ave_new, amount_new, exec_time_ns_or_None).
    """
    from concourse.bass_utils import run_bass_kernel_spmd

    nc = _build()
    in_maps = [_prep_core_inputs(features, labels, c) for c in range(N_CORES)]
    res = run_bass_kernel_spmd(
        nc,
        in_maps,
        list(range(N_CORES)),
        trace=trace,
        tmpdir=tmpdir,
    )
    parts = [res.results[c]["part_out"] for c in range(N_CORES)]
    cov_new, ave_new, amount_new = _host_combine(parts, labels, cov, ave, amt)
    return cov_new, ave_new, amount_new, res.exec_time_ns


def _worker_main(argv):
    in_path, out_path = argv[0], argv[1]
    trace = "--trace" in argv
    dat = np.load(in_path)
    cov, ave, amt, exec_ns = _run_on_device(
        dat["features"],
        dat["labels"],
        dat["cov"],
        dat["ave"],
        dat["amt"],
        trace=trace,
        tmpdir=(argv[argv.index("--trace") + 1] if trace else None),
    )
    np.savez(
        out_path,
        cov=cov,
        ave=ave,
        amt=amt,
        exec_ns=np.int64(exec_ns if exec_ns is not None else -1),
    )


def kernel(features, labels, CoVariance, Ave, Amount):
    features = np.ascontiguousarray(np.asarray(features), dtype=np.float32)
    labels = np.ascontiguousarray(np.asarray(labels), dtype=np.int64)
    cov = np.ascontiguousarray(np.asarray(CoVariance), dtype=np.float32)
    ave = np.ascontiguousarray(np.asarray(Ave), dtype=np.float32)
    amt = np.ascontiguousarray(np.asarray(Amount), dtype=np.float32)

    with tempfile.TemporaryDirectory() as td:
        in_path = os.path.join(td, "in.npz")
        out_path = os.path.join(td, "out.npz")
        np.savez(in_path, features=features, labels=labels, cov=cov, ave=ave, amt=amt)
        env = dict(os.environ)
        env.pop("JAX_PLATFORMS", None)  # worker needs the axon NC devices
        subprocess.run(
            [sys.executable, os.path.abspath(__file__), "--_worker", in_path, out_path],
            check=True,
            env=env,
        )
        out = np.load(out_path)
        return out["cov"], out["ave"], out["amt"]


if __name__ == "__main__":
    if len(sys.argv) > 1 and sys.argv[1] == "--_worker":
        _worker_main(sys.argv[2:])
    else:
        sys.exit("usage: kernel.py --_worker IN OUT [--trace DIR]")


# revision 22
# speedup vs baseline: 1.3949x; 1.2034x over previous
"""Trainium2 Bass kernel for nn_EstimatorCV (segment_reduce, 8 NeuronCores).

Data-parallel over N: each of the 8 cores reads a 16384-row shard of
`features` and computes per-class partial sums (sum and sum-of-squares)
with one-hot fp32r matmuls on the TensorEngine, accumulated in PSUM.
The per-core [20, 512] partials are summed on the host together with a
`bincount` of the labels (the per-class counts), followed by the tiny
[C, A] EMA combine — microseconds of host work, which avoids the
~40 us on-device AllReduce floor for a 39 KB collective.

Device layout per core (16384 rows = 8 groups x 16 row-tiles x 128):
  - features are host-reordered so each group is one contiguous-per-
    partition ~2 MiB DMA,
  - ACT squares each group, DVE rounds features to fp32r; both land
    interleaved in a combo tile so each row-tile needs ONE [128,20] x
    [128,512] matmul (psum[c] += onehot^T @ [x | x^2]),
  - one-hots for all 128 row-tiles are built by a single iota +
    is_equal pair.

The device work runs in a subprocess with a clean JAX environment so
the caller's JAX state (e.g. JAX_PLATFORMS=cpu) cannot break the PJRT
path.
"""

import os
import subprocess
import sys
import tempfile

import numpy as np

N_CORES = 8
N, A, C = 131072, 256, 19
RPC = N // N_CORES  # rows per core = 16384
P = 128  # SBUF partitions / rows per matmul tile
CP = 20  # classes padded to even (fp32r constraint)
SUB = 16  # row-tiles per DMA group
CHUNK = 4  # row-tiles per square/round chunk
GROUPS = RPC // (P * SUB)  # 8 groups of 2 MiB per core
GCOLS = SUB * A  # 4096 f32 per partition per group
N_TILES = GROUPS * SUB  # 128 row-tiles per core
W2 = 2 * A  # 512: [feat | feat^2] matmul width


def _prep_core_inputs(features, labels, core):
    """Build the in_map for one core from the full inputs."""
    sh = features[core * RPC : (core + 1) * RPC]
    # [RPC, A] -> [GROUPS, P, SUB*A] with rows interleaved so that
    # group g, partition p, subtile j holds row g*P*SUB + j*P + p.
    f = (
        np.ascontiguousarray(sh, dtype=np.float16)
        .reshape(GROUPS, SUB, P, A)
        .transpose(0, 2, 1, 3)
        .reshape(GROUPS, P, GCOLS)
    )
    lt = (
        labels[core * RPC : (core + 1) * RPC]
        .reshape(N_TILES, P)
        .T.astype(np.float32)
    )
    return {
        "features": np.ascontiguousarray(f),
        "labels": np.ascontiguousarray(lt),
    }


def _build():
    """Construct the Bass/Tile program (one SPMD NeuronCore view)."""
    from contextlib import ExitStack

    import concourse.mybir as mybir
    import concourse.tile as tile
    from concourse import bacc

    f32 = mybir.dt.float32
    f16 = mybir.dt.float16
    nc = bacc.Bacc(trn_type="TRN2", num_devices=N_CORES)

    feats = nc.dram_tensor("features", [GROUPS, P, GCOLS], f16, kind="ExternalInput")
    labels = nc.dram_tensor("labels", [P, N_TILES], f32, kind="ExternalInput")
    part_out = nc.dram_tensor("part_out", [CP, W2], f32, kind="ExternalOutput")

    eq = mybir.AluOpType.is_equal

    with tile.TileContext(nc) as tc, ExitStack() as ctx:
        const_pool = ctx.enter_context(tc.tile_pool(name="const", bufs=1))
        feat_pool = ctx.enter_context(tc.tile_pool(name="feat", bufs=3))
        combo_pool = ctx.enter_context(tc.tile_pool(name="combo", bufs=2))
        psum_pool = ctx.enter_context(tc.tile_pool(name="psum", bufs=1, space="PSUM"))
        tail_pool = ctx.enter_context(tc.tile_pool(name="tail", bufs=1))

        # --- constants / small loads ---
        # labels go via the ACT HWDGE queue so the SP queue starts with
        # the first feature group immediately.
        labels_t = const_pool.tile([P, N_TILES], f32)
        nc.scalar.dma_start(labels_t[:], labels.ap())

        # one-hot for ALL 128 row-tiles in one iota + one is_equal:
        # oh_all[p, t*CP + c] = (labels_t[p, t] == c)
        iota_i = const_pool.tile([P, N_TILES * CP], mybir.dt.int32)
        nc.gpsimd.iota(
            iota_i[:].rearrange("p (t c) -> p t c", c=CP),
            pattern=[[0, N_TILES], [1, CP]],
            base=0,
            channel_multiplier=0,
        )
        iota_f = const_pool.tile([P, N_TILES * CP], f32)
        nc.vector.tensor_copy(iota_f[:], iota_i[:])
        oh_all = const_pool.tile([P, N_TILES * CP], f16)
        nc.vector.tensor_tensor(
            out=oh_all[:].rearrange("p (t c) -> p t c", c=CP),
            in0=iota_f[:].rearrange("p (t c) -> p t c", c=CP),
            in1=labels_t[:].unsqueeze(2).to_broadcast([P, N_TILES, CP]),
            op=eq,
        )

        # --- main loop: psum[c, :] += onehot_j^T @ [x_j | x_j^2] ---
        psum = psum_pool.tile([CP, W2], f32)

        for g in range(GROUPS):
            feat = feat_pool.tile([P, GCOLS], f16)
            nc.sync.dma_start(feat[:], feats.ap()[g])
            combo = combo_pool.tile([P, SUB * W2], f16)
            combo_v = combo[:].rearrange("p (j x) -> p j x", x=W2)
            feat_v = feat[:].rearrange("p (j a) -> p j a", a=A)
            for c in range(SUB // CHUNK):
                cs = slice(c * CHUNK, (c + 1) * CHUNK)
                # features into the left half of each row-tile's combo
                # slot; squares (ACT/DVE alternating) into the right.
                nc.vector.tensor_copy(combo_v[:, cs, 0:A], feat_v[:, cs, :])
                if c % 2 == 0:
                    nc.scalar.square(combo_v[:, cs, A:W2], feat_v[:, cs, :])
                else:
                    nc.vector.tensor_tensor(
                        out=combo_v[:, cs, A:W2],
                        in0=feat_v[:, cs, :],
                        in1=feat_v[:, cs, :],
                        op=mybir.AluOpType.mult,
                    )
                for j in range(c * CHUNK, (c + 1) * CHUNK):
                    it = g * SUB + j
                    nc.tensor.matmul(
                        psum[:],
                        lhsT=oh_all[:, it * CP : (it + 1) * CP],
                        rhs=combo_v[:, j, :],
                        start=(it == 0),
                        stop=(it == N_TILES - 1),
                    )

        # --- write the per-core partials; reduce + EMA happen on host ---
        part = tail_pool.tile([CP, W2], f32)
        nc.vector.tensor_copy(part[:], psum[:])
        nc.sync.dma_start(part_out.ap(), part[:])

    nc.compile()
    return nc


def _host_combine(parts, labels, cov, ave_in, amt):
    """8-way partial reduce + count + EMA combine (tiny, on host)."""
    red = np.sum(np.asarray(parts, dtype=np.float64), axis=0)[:C]
    s = red[:, 0:A].astype(np.float32)
    sq = red[:, A:W2].astype(np.float32)
    cnt = np.bincount(np.asarray(labels).ravel(), minlength=C).astype(np.float32)[:C]
    cnt_c = np.maximum(cnt, 1.0)[:, None]
    ave = s / cnt_c
    var = (sq - 2.0 * ave * s + cnt[:, None] * ave * ave) / cnt_c
    denom = cnt + amt
    w = np.where(denom > 0, cnt / np.where(denom > 0, denom, 1.0), 0.0)[:, None]
    additional = w * (1.0 - w) * (ave_in - ave) ** 2
    cov_new = (cov * (1.0 - w) + var * w + additional).astype(np.float32)
    ave_new = (ave_in * (1.0 - w) + ave * w).astype(np.float32)
    amount_new = (amt + cnt).astype(np.float32)
    return cov_new, ave_new, amount_new


def _run_on_device(features, labels, cov, ave, amt, trace=False, tmpdir=None):
    """Shard inputs, compile + execute on 8 NeuronCores, return outputs.

    Must run in an interpreter whose JAX sees the axon NeuronCore devices.
    Returns (cov_new, ave_new, amount_new, exec_time_ns_or_None).
    """
    from concourse.bass_utils import run_bass_kernel_spmd

    nc = _build()
    in_maps = [_prep_core_inputs(features, labels, c) for c in range(N_CORES)]
    res = run_bass_kernel_spmd(
        nc,
        in_maps,
        list(range(N_CORES)),
        trace=trace,
        tmpdir=tmpdir,
    )
    parts = [res.results[c]["part_out"] for c in range(N_CORES)]
    cov_new, ave_new, amount_new = _host_combine(parts, labels, cov, ave, amt)
    return cov_new, ave_new, amount_new, res.exec_time_ns


def _worker_main(argv):
    in_path, out_path = argv[0], argv[1]
    trace = "--trace" in argv
    dat = np.load(in_path)
    cov, ave, amt, exec_ns = _run_on_device(
        dat["features"],
        dat["labels"],
        dat["cov"],
        dat["ave"],
        dat["amt"],
        trace=trace,
        tmpdir=(argv[argv.index("--trace") + 1] if trace else None),
    )
    np.savez(
        out_path,
        cov=cov,
        ave=ave,
        amt=amt,
        exec_ns=np.int64(exec_ns if exec_ns is not None else -1),
    )


def kernel(features, labels, CoVariance, Ave, Amount):
    features = np.ascontiguousarray(np.asarray(features), dtype=np.float32)
    labels = np.ascontiguousarray(np.asarray(labels), dtype=np.int64)
    cov = np.ascontiguousarray(np.asarray(CoVariance), dtype=np.float32)
    ave = np.ascontiguousarray(np.asarray(Ave), dtype=np.float32)
    amt = np.ascontiguousarray(np.asarray(Amount), dtype=np.float32)

    with tempfile.TemporaryDirectory() as td:
        in_path = os.path.join(td, "in.npz")
        out_path = os.path.join(td, "out.npz")
        np.savez(in_path, features=features, labels=labels, cov=cov, ave=ave, amt=amt)
        env = dict(os.environ)
        env.pop("JAX_PLATFORMS", None)  # worker needs the axon NC devices
        subprocess.run(
            [sys.executable, os.path.abspath(__file__), "--_worker", in_path, out_path],
            check=True,
            env=env,
        )
        out = np.load(out_path)
        return out["cov"], out["ave"], out["amt"]


if __name__ == "__main__":
    if len(sys.argv) > 1 and sys.argv[1] == "--_worker":
        _worker_main(sys.argv[2:])
    else:
        sys.exit("usage: kernel.py --_worker IN OUT [--trace DIR]")


# revision 28
# speedup vs baseline: 1.4238x; 1.0207x over previous
"""Trainium2 Bass kernel for nn_EstimatorCV (segment_reduce, 8 NeuronCores).

Data-parallel over N: each of the 8 cores reads a 16384-row shard of
`features` and computes per-class partial sums (sum and sum-of-squares)
with one-hot fp32r matmuls on the TensorEngine, accumulated in PSUM.
The per-core [20, 512] partials are summed on the host together with a
`bincount` of the labels (the per-class counts), followed by the tiny
[C, A] EMA combine — microseconds of host work, which avoids the
~40 us on-device AllReduce floor for a 39 KB collective.

Device layout per core (16384 rows = 8 groups x 16 row-tiles x 128):
  - features are host-reordered so each group is one contiguous-per-
    partition ~2 MiB DMA,
  - ACT squares each group, DVE rounds features to fp32r; both land
    interleaved in a combo tile so each row-tile needs ONE [128,20] x
    [128,512] matmul (psum[c] += onehot^T @ [x | x^2]),
  - one-hots for all 128 row-tiles are built by a single iota +
    is_equal pair.

The device work runs in a subprocess with a clean JAX environment so
the caller's JAX state (e.g. JAX_PLATFORMS=cpu) cannot break the PJRT
path.
"""

import os
import subprocess
import sys
import tempfile

import numpy as np

N_CORES = 8
N, A, C = 131072, 256, 19
RPC = N // N_CORES  # rows per core = 16384
P = 128  # SBUF partitions / rows per matmul tile
CP = 20  # classes padded to even (fp32r constraint)
SUB = 16  # row-tiles per DMA group
CHUNK = 4  # row-tiles per square/round chunk
GROUPS = RPC // (P * SUB)  # 8 groups of 2 MiB per core
GCOLS = SUB * A  # 4096 f32 per partition per group
N_TILES = GROUPS * SUB  # 128 row-tiles per core
W2 = 2 * A  # 512: [feat | feat^2] matmul width


def _prep_core_inputs(features, labels, core):
    """Build the in_map for one core from the full inputs."""
    sh = features[core * RPC : (core + 1) * RPC]
    # [RPC, A] -> [GROUPS, P, SUB*A] with rows interleaved so that
    # group g, partition p, subtile j holds row g*P*SUB + j*P + p.
    f = (
        np.ascontiguousarray(sh, dtype=np.float16)
        .reshape(GROUPS, SUB, P, A)
        .transpose(0, 2, 1, 3)
        .reshape(GROUPS, P, GCOLS)
    )
    lt = (
        labels[core * RPC : (core + 1) * RPC]
        .reshape(N_TILES, P)
        .T.astype(np.float32)
    )
    return {
        "features": np.ascontiguousarray(f),
        "labels": np.ascontiguousarray(lt),
    }


def _build():
    """Construct the Bass/Tile program (one SPMD NeuronCore view)."""
    from contextlib import ExitStack

    import concourse.mybir as mybir
    import concourse.tile as tile
    from concourse import bacc

    f32 = mybir.dt.float32
    f16 = mybir.dt.float16
    nc = bacc.Bacc(trn_type="TRN2", num_devices=N_CORES)

    feats = nc.dram_tensor("features", [GROUPS, P, GCOLS], f16, kind="ExternalInput")
    labels = nc.dram_tensor("labels", [P, N_TILES], f32, kind="ExternalInput")
    part_out = nc.dram_tensor("part_out", [P, W2], f32, kind="ExternalOutput")

    eq = mybir.AluOpType.is_equal

    with tile.TileContext(nc) as tc, ExitStack() as ctx:
        const_pool = ctx.enter_context(tc.tile_pool(name="const", bufs=1))
        feat_pool = ctx.enter_context(tc.tile_pool(name="feat", bufs=3))
        combo_pool = ctx.enter_context(tc.tile_pool(name="combo", bufs=2))
        psum_pool = ctx.enter_context(tc.tile_pool(name="psum", bufs=1, space="PSUM"))
        tail_pool = ctx.enter_context(tc.tile_pool(name="tail", bufs=1))

        # --- constants / small loads ---
        # labels go via the ACT HWDGE queue so the SP queue starts with
        # the first feature group immediately.
        labels_t = const_pool.tile([P, N_TILES], f32)
        nc.scalar.dma_start(labels_t[:], labels.ap())

        # one-hot for ALL 128 row-tiles in one iota + one is_equal:
        # oh_all[p, t*CP + c] = (labels_t[p, t] == c)
        iota_i = const_pool.tile([P, N_TILES * CP], mybir.dt.int32)
        nc.gpsimd.iota(
            iota_i[:].rearrange("p (t c) -> p t c", c=CP),
            pattern=[[0, N_TILES], [1, CP]],
            base=0,
            channel_multiplier=0,
        )
        iota_f = const_pool.tile([P, N_TILES * CP], f32)
        nc.vector.tensor_copy(iota_f[:], iota_i[:])
        oh_all = const_pool.tile([P, N_TILES * CP], f16)
        nc.vector.tensor_tensor(
            out=oh_all[:].rearrange("p (t c) -> p t c", c=CP),
            in0=iota_f[:].rearrange("p (t c) -> p t c", c=CP),
            in1=labels_t[:].unsqueeze(2).to_broadcast([P, N_TILES, CP]),
            op=eq,
        )

        # --- main loop: psum[32b+c, b, :] += onehot_j^T @ [x_j | x_j^2] ---
        # 4-way PE column-group packing: row-tile j lands in column group
        # b = j%4 (tile_position=(0,32b), out partitions 32b..32b+19), so
        # 4 matmuls stream concurrently through separate XBUSes. Each
        # block accumulates in its own PSUM bank.
        psum = psum_pool.tile([P, 4 * W2], f32)
        psum_v = psum[:].rearrange("p (b x) -> p b x", x=W2)

        for g in range(GROUPS):
            feat = feat_pool.tile([P, GCOLS], f16)
            if g == 0:
                # chunked first load so the PE pipeline starts sooner
                for c in range(SUB // CHUNK):
                    nc.sync.dma_start(
                        feat[:, c * CHUNK * A : (c + 1) * CHUNK * A],
                        feats.ap()[g][:, c * CHUNK * A : (c + 1) * CHUNK * A],
                    )
            else:
                nc.sync.dma_start(feat[:], feats.ap()[g])
            combo = combo_pool.tile([P, SUB * W2], f16)
            combo_v = combo[:].rearrange("p (j x) -> p j x", x=W2)
            feat_v = feat[:].rearrange("p (j a) -> p j a", a=A)
            for c in range(SUB // CHUNK):
                cs = slice(c * CHUNK, (c + 1) * CHUNK)
                # features into the left half of each row-tile's combo
                # slot; squares (ACT/DVE alternating) into the right.
                nc.vector.tensor_copy(combo_v[:, cs, 0:A], feat_v[:, cs, :])
                if c % 2 == 0:
                    nc.scalar.square(combo_v[:, cs, A:W2], feat_v[:, cs, :])
                else:
                    nc.vector.tensor_tensor(
                        out=combo_v[:, cs, A:W2],
                        in0=feat_v[:, cs, :],
                        in1=feat_v[:, cs, :],
                        op=mybir.AluOpType.mult,
                    )
                for j in range(c * CHUNK, (c + 1) * CHUNK):
                    it = g * SUB + j
                    b = it % 4
                    nc.tensor.matmul(
                        psum_v[32 * b : 32 * b + CP, b, :],
                        lhsT=oh_all[:, it * CP : (it + 1) * CP],
                        rhs=combo_v[:, j, :],
                        start=(it < 4),
                        stop=(it >= N_TILES - 4),
                        tile_position=(0, 32 * b),
                    )

        # --- write the per-core partials; reduce + EMA happen on host ---
        part = tail_pool.tile([P, W2], f32)
        nc.gpsimd.memset(part[:], 0.0)
        for b in range(4):
            nc.vector.tensor_copy(
                part[32 * b : 32 * b + CP, :], psum_v[32 * b : 32 * b + CP, b, :]
            )
        nc.sync.dma_start(part_out.ap(), part[:])

    nc.compile()
    return nc


def _host_combine(parts, labels, cov, ave_in, amt):
    """8-way partial reduce + count + EMA combine (tiny, on host)."""
    acc = np.sum(np.asarray(parts, dtype=np.float64), axis=0)  # [128, W2]
    # sum the 4 column-group blocks (partitions 32b .. 32b+CP)
    red = sum(acc[32 * b : 32 * b + CP] for b in range(4))[:C]
    s = red[:, 0:A].astype(np.float32)
    sq = red[:, A:W2].astype(np.float32)
    cnt = np.bincount(np.asarray(labels).ravel(), minlength=C).astype(np.float32)[:C]
    cnt_c = np.maximum(cnt, 1.0)[:, None]
    ave = s / cnt_c
    var = (sq - 2.0 * ave * s + cnt[:, None] * ave * ave) / cnt_c
    denom = cnt + amt
    w = np.where(denom > 0, cnt / np.where(denom > 0, denom, 1.0), 0.0)[:, None]
    additional = w * (1.0 - w) * (ave_in - ave) ** 2
    cov_new = (cov * (1.0 - w) + var * w + additional).astype(np.float32)
    ave_new = (ave_in * (1.0 - w) + ave * w).astype(np.float32)
    amount_new = (amt + cnt).astype(np.float32)
    return cov_new, ave_new, amount_new


def _run_on_device(features, labels, cov, ave, amt, trace=False, tmpdir=None):
    """Shard inputs, compile + execute on 8 NeuronCores, return outputs.

    Must run in an interpreter whose JAX sees the axon NeuronCore devices.
    Returns (cov_new, ave_new, amount_new, exec_time_ns_or_None).
    """
    from concourse.bass_utils import run_bass_kernel_spmd

    nc = _build()
    in_maps = [_prep_core_inputs(features, labels, c) for c in range(N_CORES)]
    res = run_bass_kernel_spmd(
        nc,
        in_maps,
        list(range(N_CORES)),
        trace=trace,
        tmpdir=tmpdir,
    )
    parts = [res.results[c]["part_out"] for c in range(N_CORES)]
    cov_new, ave_new, amount_new = _host_combine(parts, labels, cov, ave, amt)
    return cov_new, ave_new, amount_new, res.exec_time_ns


def _worker_main(argv):
    in_path, out_path = argv[0], argv[1]
    trace = "--trace" in argv
    dat = np.load(in_path)
    cov, ave, amt, exec_ns = _run_on_device(
        dat["features"],
        dat["labels"],
        dat["cov"],
        dat["ave"],
        dat["amt"],
        trace=trace,
        tmpdir=(argv[argv.index("--trace") + 1] if trace else None),
    )
    np.savez(
        out_path,
        cov=cov,
        ave=ave,
        amt=amt,
        exec_ns=np.int64(exec_ns if exec_ns is not None else -1),
    )


def kernel(features, labels, CoVariance, Ave, Amount):
    features = np.ascontiguousarray(np.asarray(features), dtype=np.float32)
    labels = np.ascontiguousarray(np.asarray(labels), dtype=np.int64)
    cov = np.ascontiguousarray(np.asarray(CoVariance), dtype=np.float32)
    ave = np.ascontiguousarray(np.asarray(Ave), dtype=np.float32)
    amt = np.ascontiguousarray(np.asarray(Amount), dtype=np.float32)

    with tempfile.TemporaryDirectory() as td:
        in_path = os.path.join(td, "in.npz")
        out_path = os.path.join(td, "out.npz")
        np.savez(in_path, features=features, labels=labels, cov=cov, ave=ave, amt=amt)
        env = dict(os.environ)
        env.pop("JAX_PLATFORMS", None)  # worker needs the axon NC devices
        subprocess.run(
            [sys.executable, os.path.abspath(__file__), "--_worker", in_path, out_path],
            check=True,
            env=env,
        )
        out = np.load(out_path)
        return out["cov"], out["ave"], out["amt"]


if __name__ == "__main__":
    if len(sys.argv) > 1 and sys.argv[1] == "--_worker":
        _worker_main(sys.argv[2:])
    else:
        sys.exit("usage: kernel.py --_worker IN OUT [--trace DIR]")


# revision 31
# speedup vs baseline: 1.4471x; 1.0164x over previous
"""Trainium2 Bass kernel for nn_EstimatorCV (segment_reduce, 8 NeuronCores).

Data-parallel over N: each of the 8 cores reads a 16384-row shard of
`features` and computes per-class partial sums (sum and sum-of-squares)
with one-hot fp32r matmuls on the TensorEngine, accumulated in PSUM.
The per-core [20, 512] partials are summed on the host together with a
`bincount` of the labels (the per-class counts), followed by the tiny
[C, A] EMA combine — microseconds of host work, which avoids the
~40 us on-device AllReduce floor for a 39 KB collective.

Device layout per core (16384 rows = 8 groups x 16 row-tiles x 128):
  - features are host-reordered so each group is one contiguous-per-
    partition ~2 MiB DMA,
  - ACT squares each group, DVE rounds features to fp32r; both land
    interleaved in a combo tile so each row-tile needs ONE [128,20] x
    [128,512] matmul (psum[c] += onehot^T @ [x | x^2]),
  - one-hots for all 128 row-tiles are built by a single iota +
    is_equal pair.

The device work runs in a subprocess with a clean JAX environment so
the caller's JAX state (e.g. JAX_PLATFORMS=cpu) cannot break the PJRT
path.
"""

import os
import subprocess
import sys
import tempfile

import numpy as np

N_CORES = 8
N, A, C = 131072, 256, 19
RPC = N // N_CORES  # rows per core = 16384
P = 128  # SBUF partitions / rows per matmul tile
CP = 20  # classes padded to even (fp32r constraint)
SUB = 16  # row-tiles per DMA group
CHUNK = 4  # row-tiles per square/round chunk
GROUPS = RPC // (P * SUB)  # 8 groups of 2 MiB per core
GCOLS = SUB * A  # 4096 f32 per partition per group
N_TILES = GROUPS * SUB  # 128 row-tiles per core
W2 = 2 * A  # 512: [feat | feat^2] matmul width


def _prep_core_inputs(features, labels, core):
    """Build the in_map for one core from the full inputs."""
    sh = features[core * RPC : (core + 1) * RPC]
    # [RPC, A] -> [GROUPS, P, SUB*A] with rows interleaved so that
    # group g, partition p, subtile j holds row g*P*SUB + j*P + p.
    f = (
        np.ascontiguousarray(sh, dtype=np.float16)
        .reshape(GROUPS, SUB, P, A)
        .transpose(0, 2, 1, 3)
        .reshape(GROUPS, P, GCOLS)
    )
    lt = (
        labels[core * RPC : (core + 1) * RPC]
        .reshape(N_TILES, P)
        .T.astype(np.float32)
    )
    return {
        "features": np.ascontiguousarray(f),
        "labels": np.ascontiguousarray(lt),
    }


def _build():
    """Construct the Bass/Tile program (one SPMD NeuronCore view)."""
    from contextlib import ExitStack

    import concourse.mybir as mybir
    import concourse.tile as tile
    from concourse import bacc

    f32 = mybir.dt.float32
    f16 = mybir.dt.float16
    nc = bacc.Bacc(trn_type="TRN2", num_devices=N_CORES)

    feats = nc.dram_tensor("features", [GROUPS, P, GCOLS], f16, kind="ExternalInput")
    labels = nc.dram_tensor("labels", [P, N_TILES], f32, kind="ExternalInput")
    part_out = nc.dram_tensor("part_out", [P, W2], f32, kind="ExternalOutput")

    eq = mybir.AluOpType.is_equal

    with tile.TileContext(nc) as tc, ExitStack() as ctx:
        const_pool = ctx.enter_context(tc.tile_pool(name="const", bufs=1))
        combo_pool = ctx.enter_context(tc.tile_pool(name="combo", bufs=3))
        psum_pool = ctx.enter_context(tc.tile_pool(name="psum", bufs=1, space="PSUM"))
        tail_pool = ctx.enter_context(tc.tile_pool(name="tail", bufs=1))

        # --- constants / small loads ---
        # labels go via the ACT HWDGE queue so the SP queue starts with
        # the first feature group immediately.
        labels_t = const_pool.tile([P, N_TILES], f32)
        nc.scalar.dma_start(labels_t[:], labels.ap())

        # one-hot for ALL 128 row-tiles in one iota + one is_equal:
        # oh_all[p, t*CP + c] = (labels_t[p, t] == c)
        iota_i = const_pool.tile([P, N_TILES * CP], mybir.dt.int32)
        nc.gpsimd.iota(
            iota_i[:].rearrange("p (t c) -> p t c", c=CP),
            pattern=[[0, N_TILES], [1, CP]],
            base=0,
            channel_multiplier=0,
        )
        iota_f = const_pool.tile([P, N_TILES * CP], f32)
        nc.vector.tensor_copy(iota_f[:], iota_i[:])
        oh_all = const_pool.tile([P, N_TILES * CP], f16)
        nc.vector.tensor_tensor(
            out=oh_all[:].rearrange("p (t c) -> p t c", c=CP),
            in0=iota_f[:].rearrange("p (t c) -> p t c", c=CP),
            in1=labels_t[:].unsqueeze(2).to_broadcast([P, N_TILES, CP]),
            op=eq,
        )

        # --- main loop: psum[32b+c, b, :] += onehot_j^T @ [x_j | x_j^2] ---
        # 4-way PE column-group packing: row-tile j lands in column group
        # b = j%4 (tile_position=(0,32b), out partitions 32b..32b+19), so
        # 4 matmuls stream concurrently through separate XBUSes. Each
        # block accumulates in its own PSUM bank.
        psum = psum_pool.tile([P, 4 * W2], f32)
        psum_v = psum[:].rearrange("p (b x) -> p b x", x=W2)

        CCH = CHUNK * A  # 1024: columns per square-chunk
        for g in range(GROUPS):
            # combo: [features(4096) | squares(4096)]; the DMA writes the
            # feature half directly (contiguous per partition), ACT/DVE
            # fill the square half; the matmul reads a 2-block AP.
            combo = combo_pool.tile([P, 2 * GCOLS], f16)
            if g in (0, GROUPS - 1):
                # chunked first/last load: shorter pipeline fill + tail
                for c in range(SUB // CHUNK):
                    nc.sync.dma_start(
                        combo[:, c * CCH : (c + 1) * CCH],
                        feats.ap()[g][:, c * CCH : (c + 1) * CCH],
                    )
            else:
                nc.sync.dma_start(combo[:, 0:GCOLS], feats.ap()[g])
            combo_v = combo[:].rearrange("p (h x) -> p h x", h=2)
            for c in range(SUB // CHUNK):
                src = combo[:, c * CCH : (c + 1) * CCH]
                dst = combo[:, GCOLS + c * CCH : GCOLS + (c + 1) * CCH]
                if c % 2 == 0:
                    nc.scalar.square(dst, src)
                else:
                    nc.vector.tensor_tensor(out=dst, in0=src, in1=src, op=mybir.AluOpType.mult)
                for j in range(c * CHUNK, (c + 1) * CHUNK):
                    it = g * SUB + j
                    b = it % 4
                    # two contiguous-rhs matmuls into the block's bank:
                    # cols 0:256 accumulate sum(x), 256:512 sum(x^2).
                    # One accumulation group per bank: start clears the
                    # whole bank's has_written, stop on the last write.
                    nc.tensor.matmul(
                        psum_v[32 * b : 32 * b + CP, b, 0:A],
                        lhsT=oh_all[:, it * CP : (it + 1) * CP],
                        rhs=combo[:, j * A : (j + 1) * A],
                        start=(it < 4),
                        stop=False,
                        skip_group_check=True,
                        tile_position=(0, 32 * b),
                    )
                    nc.tensor.matmul(
                        psum_v[32 * b : 32 * b + CP, b, A:W2],
                        lhsT=oh_all[:, it * CP : (it + 1) * CP],
                        rhs=combo[:, GCOLS + j * A : GCOLS + (j + 1) * A],
                        start=False,
                        stop=(it >= N_TILES - 4),
                        skip_group_check=True,
                        tile_position=(0, 32 * b),
                    )

        # --- write the per-core partials; reduce + EMA happen on host ---
        part = tail_pool.tile([P, W2], f32)
        nc.gpsimd.memset(part[:], 0.0)
        for b in range(4):
            nc.vector.tensor_copy(
                part[32 * b : 32 * b + CP, :], psum_v[32 * b : 32 * b + CP, b, :]
            )
        nc.sync.dma_start(part_out.ap(), part[:])

    nc.compile()
    return nc


def _host_combine(parts, labels, cov, ave_in, amt):
    """8-way partial reduce + count + EMA combine (tiny, on host)."""
    acc = np.sum(np.asarray(parts, dtype=np.float64), axis=0)  # [128, W2]
    # sum the 4 column-group blocks (partitions 32b .. 32b+CP)
    red = sum(acc[32 * b : 32 * b + CP] for b in range(4))[:C]
    s = red[:, 0:A].astype(np.float32)
    sq = red[:, A:W2].astype(np.float32)
    cnt = np.bincount(np.asarray(labels).ravel(), minlength=C).astype(np.float32)[:C]
    cnt_c = np.maximum(cnt, 1.0)[:, None]
    ave = s / cnt_c
    var = (sq - 2.0 * ave * s + cnt[:, None] * ave * ave) / cnt_c
    denom = cnt + amt
    w = np.where(denom > 0, cnt / np.where(denom > 0, denom, 1.0), 0.0)[:, None]
    additional = w * (1.0 - w) * (ave_in - ave) ** 2
    cov_new = (cov * (1.0 - w) + var * w + additional).astype(np.float32)
    ave_new = (ave_in * (1.0 - w) + ave * w).astype(np.float32)
    amount_new = (amt + cnt).astype(np.float32)
    return cov_new, ave_new, amount_new


def _run_on_device(features, labels, cov, ave, amt, trace=False, tmpdir=None):
    """Shard inputs, compile + execute on 8 NeuronCores, return outputs.

    Must run in an interpreter whose JAX sees the axon NeuronCore devices.
    Returns (cov_new, ave_new, amount_new, exec_time_ns_or_None).
    """
    from concourse.bass_utils import run_bass_kernel_spmd

    nc = _build()
    in_maps = [_prep_core_inputs(features, labels, c) for c in range(N_CORES)]
    res = run_bass_kernel_spmd(
        nc,
        in_maps,
        list(range(N_CORES)),
        trace=trace,
        tmpdir=tmpdir,
    )
    parts = [res.results[c]["part_out"] for c in range(N_CORES)]
    cov_new, ave_new, amount_new = _host_combine(parts, labels, cov, ave, amt)
    return cov_new, ave_new, amount_new, res.exec_time_ns


def _worker_main(argv):
    in_path, out_path = argv[0], argv[1]
    trace = "--trace" in argv
    dat = np.load(in_path)
    cov, ave, amt, exec_ns = _run_on_device(
        dat["features"],
        dat["labels"],
        dat["cov"],
        dat["ave"],
        dat["amt"],
        trace=trace,
        tmpdir=(argv[argv.index("--trace") + 1] if trace else None),
    )
    np.savez(
        out_path,
        cov=cov,
        ave=ave,
        amt=amt,
        exec_ns=np.int64(exec_ns if exec_ns is not None else -1),
    )


def kernel(features, labels, CoVariance, Ave, Amount):
    features = np.ascontiguousarray(np.asarray(features), dtype=np.float32)
    labels = np.ascontiguousarray(np.asarray(labels), dtype=np.int64)
    cov = np.ascontiguousarray(np.asarray(CoVariance), dtype=np.float32)
    ave = np.ascontiguousarray(np.asarray(Ave), dtype=np.float32)
    amt = np.ascontiguousarray(np.asarray(Amount), dtype=np.float32)

    with tempfile.TemporaryDirectory() as td:
        in_path = os.path.join(td, "in.npz")
        out_path = os.path.join(td, "out.npz")
        np.savez(in_path, features=features, labels=labels, cov=cov, ave=ave, amt=amt)
        env = dict(os.environ)
        env.pop("JAX_PLATFORMS", None)  # worker needs the axon NC devices
        subprocess.run(
            [sys.executable, os.path.abspath(__file__), "--_worker", in_path, out_path],
            check=True,
            env=env,
        )
        out = np.load(out_path)
        return out["cov"], out["ave"], out["amt"]


if __name__ == "__main__":
    if len(sys.argv) > 1 and sys.argv[1] == "--_worker":
        _worker_main(sys.argv[2:])
    else:
        sys.exit("usage: kernel.py --_worker IN OUT [--trace DIR]")


# revision 32
# speedup vs baseline: 1.6532x; 1.1424x over previous
"""Trainium2 Bass kernel for nn_EstimatorCV (segment_reduce, 8 NeuronCores).

Data-parallel over N: each of the 8 cores reads a 16384-row shard of
`features` and computes per-class partial sums (sum and sum-of-squares)
with one-hot fp32r matmuls on the TensorEngine, accumulated in PSUM.
The per-core [20, 512] partials are summed on the host together with a
`bincount` of the labels (the per-class counts), followed by the tiny
[C, A] EMA combine — microseconds of host work, which avoids the
~40 us on-device AllReduce floor for a 39 KB collective.

Device layout per core (16384 rows = 8 groups x 16 row-tiles x 128):
  - features are host-reordered so each group is one contiguous-per-
    partition ~2 MiB DMA,
  - ACT squares each group, DVE rounds features to fp32r; both land
    interleaved in a combo tile so each row-tile needs ONE [128,20] x
    [128,512] matmul (psum[c] += onehot^T @ [x | x^2]),
  - one-hots for all 128 row-tiles are built by a single iota +
    is_equal pair.

The device work runs in a subprocess with a clean JAX environment so
the caller's JAX state (e.g. JAX_PLATFORMS=cpu) cannot break the PJRT
path.
"""

import os
import subprocess
import sys
import tempfile

import numpy as np

N_CORES = 8
N, A, C = 131072, 256, 19
RPC = N // N_CORES  # rows per core = 16384
P = 128  # SBUF partitions / rows per matmul tile
CP = 20  # classes padded to even (fp32r constraint)
SUB = 16  # row-tiles per DMA group
CHUNK = 4  # row-tiles per square/round chunk
GROUPS = RPC // (P * SUB)  # 8 groups of 2 MiB per core
GCOLS = SUB * A  # 4096 f32 per partition per group
N_TILES = GROUPS * SUB  # 128 row-tiles per core
W2 = 2 * A  # 512: [feat | feat^2] matmul width


def _prep_core_inputs(features, labels, core):
    """Build the in_map for one core from the full inputs."""
    sh = features[core * RPC : (core + 1) * RPC]
    # [RPC, A] -> [GROUPS, P, SUB*A] with rows interleaved so that
    # group g, partition p, subtile j holds row g*P*SUB + j*P + p.
    f = (
        np.ascontiguousarray(sh, dtype=np.float16)
        .reshape(GROUPS, SUB, P, A)
        .transpose(0, 2, 1, 3)
        .reshape(GROUPS, P, GCOLS)
    )
    lt = (
        labels[core * RPC : (core + 1) * RPC]
        .reshape(N_TILES, P)
        .T.astype(np.float32)
    )
    return {
        "features": np.ascontiguousarray(f),
        "labels": np.ascontiguousarray(lt),
    }


def _build():
    """Construct the Bass/Tile program (one SPMD NeuronCore view)."""
    from contextlib import ExitStack

    import concourse.mybir as mybir
    import concourse.tile as tile
    from concourse import bacc

    f32 = mybir.dt.float32
    f16 = mybir.dt.float16
    nc = bacc.Bacc(trn_type="TRN2", num_devices=N_CORES)

    feats = nc.dram_tensor("features", [GROUPS, P, GCOLS], f16, kind="ExternalInput")
    labels = nc.dram_tensor("labels", [P, N_TILES], f32, kind="ExternalInput")
    part_out = nc.dram_tensor("part_out", [P, W2], f32, kind="ExternalOutput")

    eq = mybir.AluOpType.is_equal

    with tile.TileContext(nc) as tc, ExitStack() as ctx:
        const_pool = ctx.enter_context(tc.tile_pool(name="const", bufs=1))
        combo_pool = ctx.enter_context(tc.tile_pool(name="combo", bufs=4))
        psum_pool = ctx.enter_context(tc.tile_pool(name="psum", bufs=1, space="PSUM"))
        tail_pool = ctx.enter_context(tc.tile_pool(name="tail", bufs=1))

        # --- constants / small loads ---
        # labels go via the ACT HWDGE queue so the SP queue starts with
        # the first feature group immediately.
        labels_t = const_pool.tile([P, N_TILES], f32)
        nc.scalar.dma_start(labels_t[:], labels.ap())

        # one-hot for ALL 128 row-tiles in one iota + one is_equal:
        # oh_all[p, t*CP + c] = (labels_t[p, t] == c)
        iota_i = const_pool.tile([P, N_TILES * CP], mybir.dt.int32)
        nc.gpsimd.iota(
            iota_i[:].rearrange("p (t c) -> p t c", c=CP),
            pattern=[[0, N_TILES], [1, CP]],
            base=0,
            channel_multiplier=0,
        )
        iota_f = const_pool.tile([P, N_TILES * CP], f32)
        nc.vector.tensor_copy(iota_f[:], iota_i[:])
        oh_all = const_pool.tile([P, N_TILES * CP], f16)
        nc.vector.tensor_tensor(
            out=oh_all[:].rearrange("p (t c) -> p t c", c=CP),
            in0=iota_f[:].rearrange("p (t c) -> p t c", c=CP),
            in1=labels_t[:].unsqueeze(2).to_broadcast([P, N_TILES, CP]),
            op=eq,
        )

        # --- main loop: psum[32b+c, b, :] += onehot_j^T @ [x_j | x_j^2] ---
        # 4-way PE column-group packing: row-tile j lands in column group
        # b = j%4 (tile_position=(0,32b), out partitions 32b..32b+19), so
        # 4 matmuls stream concurrently through separate XBUSes. Each
        # block accumulates in its own PSUM bank.
        psum = psum_pool.tile([P, 4 * W2], f32)
        psum_v = psum[:].rearrange("p (b x) -> p b x", x=W2)

        CCH = CHUNK * A  # 1024: columns per square-chunk
        for g in range(GROUPS):
            # combo: [features(4096) | squares(4096)]; the DMA writes the
            # feature half directly (contiguous per partition), ACT/DVE
            # fill the square half; the matmul reads a 2-block AP.
            combo = combo_pool.tile([P, 2 * GCOLS], f16)
            if g == GROUPS - 1:
                # chunked last load: compute follows each chunk -> short tail
                for c in range(SUB // CHUNK):
                    nc.sync.dma_start(
                        combo[:, c * CCH : (c + 1) * CCH],
                        feats.ap()[g][:, c * CCH : (c + 1) * CCH],
                    )
            else:
                nc.sync.dma_start(combo[:, 0:GCOLS], feats.ap()[g])
            combo_v = combo[:].rearrange("p (h x) -> p h x", h=2)
            for c in range(SUB // CHUNK):
                src = combo[:, c * CCH : (c + 1) * CCH]
                dst = combo[:, GCOLS + c * CCH : GCOLS + (c + 1) * CCH]
                if c % 2 == 0:
                    nc.scalar.square(dst, src)
                else:
                    nc.vector.tensor_tensor(out=dst, in0=src, in1=src, op=mybir.AluOpType.mult)
                for j in range(c * CHUNK, (c + 1) * CHUNK):
                    it = g * SUB + j
                    b = it % 4
                    # two contiguous-rhs matmuls into the block's bank:
                    # cols 0:256 accumulate sum(x), 256:512 sum(x^2).
                    # One accumulation group per bank: start clears the
                    # whole bank's has_written, stop on the last write.
                    nc.tensor.matmul(
                        psum_v[32 * b : 32 * b + CP, b, 0:A],
                        lhsT=oh_all[:, it * CP : (it + 1) * CP],
                        rhs=combo[:, j * A : (j + 1) * A],
                        start=(it < 4),
                        stop=False,
                        skip_group_check=True,
                        tile_position=(0, 32 * b),
                    )
                    nc.tensor.matmul(
                        psum_v[32 * b : 32 * b + CP, b, A:W2],
                        lhsT=oh_all[:, it * CP : (it + 1) * CP],
                        rhs=combo[:, GCOLS + j * A : GCOLS + (j + 1) * A],
                        start=False,
                        stop=(it >= N_TILES - 4),
                        skip_group_check=True,
                        tile_position=(0, 32 * b),
                    )

        # --- write the per-core partials; reduce + EMA happen on host ---
        part = tail_pool.tile([P, W2], f32)
        nc.gpsimd.memset(part[:], 0.0)
        for b in range(4):
            src_ap = psum_v[32 * b : 32 * b + CP, b, :]
            dst_ap = part[32 * b : 32 * b + CP, :]
            if b % 2 == 0:
                nc.vector.tensor_copy(dst_ap, src_ap)
            else:
                nc.scalar.copy(dst_ap, src_ap)
        nc.sync.dma_start(part_out.ap(), part[:])

    nc.compile()
    return nc


def _host_combine(parts, labels, cov, ave_in, amt):
    """8-way partial reduce + count + EMA combine (tiny, on host)."""
    acc = np.sum(np.asarray(parts, dtype=np.float64), axis=0)  # [128, W2]
    # sum the 4 column-group blocks (partitions 32b .. 32b+CP)
    red = sum(acc[32 * b : 32 * b + CP] for b in range(4))[:C]
    s = red[:, 0:A].astype(np.float32)
    sq = red[:, A:W2].astype(np.float32)
    cnt = np.bincount(np.asarray(labels).ravel(), minlength=C).astype(np.float32)[:C]
    cnt_c = np.maximum(cnt, 1.0)[:, None]
    ave = s / cnt_c
    var = (sq - 2.0 * ave * s + cnt[:, None] * ave * ave) / cnt_c
    denom = cnt + amt
    w = np.where(denom > 0, cnt / np.where(denom > 0, denom, 1.0), 0.0)[:, None]
    additional = w * (1.0 - w) * (ave_in - ave) ** 2
    cov_new = (cov * (1.0 - w) + var * w + additional).astype(np.float32)
    ave_new = (ave_in * (1.0 - w) + ave * w).astype(np.float32)
    amount_new = (amt + cnt).astype(np.float32)
    return cov_new, ave_new, amount_new


def _run_on_device(features, labels, cov, ave, amt, trace=False, tmpdir=None):
    """Shard inputs, compile + execute on 8 NeuronCores, return outputs.

    Must run in an interpreter whose JAX sees the axon NeuronCore devices.
    Returns (cov_new, ave_new, amount_new, exec_time_ns_or_None).
    """
    from concourse.bass_utils import run_bass_kernel_spmd

    nc = _build()
    in_maps = [_prep_core_inputs(features, labels, c) for c in range(N_CORES)]
    res = run_bass_kernel_spmd(
        nc,
        in_maps,
        list(range(N_CORES)),
        trace=trace,
        tmpdir=tmpdir,
    )
    parts = [res.results[c]["part_out"] for c in range(N_CORES)]
    cov_new, ave_new, amount_new = _host_combine(parts, labels, cov, ave, amt)
    return cov_new, ave_new, amount_new, res.exec_time_ns


def _worker_main(argv):
    in_path, out_path = argv[0], argv[1]
    trace = "--trace" in argv
    dat = np.load(in_path)
    cov, ave, amt, exec_ns = _run_on_device(
        dat["features"],
        dat["labels"],
        dat["cov"],
        dat["ave"],
        dat["amt"],
        trace=trace,
        tmpdir=(argv[argv.index("--trace") + 1] if trace else None),
    )
    np.savez(
        out_path,
        cov=cov,
        ave=ave,
        amt=amt,
        exec_ns=np.int64(exec_ns if exec_ns is not None else -1),
    )


def kernel(features, labels, CoVariance, Ave, Amount):
    features = np.ascontiguousarray(np.asarray(features), dtype=np.float32)
    labels = np.ascontiguousarray(np.asarray(labels), dtype=np.int64)
    cov = np.ascontiguousarray(np.asarray(CoVariance), dtype=np.float32)
    ave = np.ascontiguousarray(np.asarray(Ave), dtype=np.float32)
    amt = np.ascontiguousarray(np.asarray(Amount), dtype=np.float32)

    with tempfile.TemporaryDirectory() as td:
        in_path = os.path.join(td, "in.npz")
        out_path = os.path.join(td, "out.npz")
        np.savez(in_path, features=features, labels=labels, cov=cov, ave=ave, amt=amt)
        env = dict(os.environ)
        env.pop("JAX_PLATFORMS", None)  # worker needs the axon NC devices
        subprocess.run(
            [sys.executable, os.path.abspath(__file__), "--_worker", in_path, out_path],
            check=True,
            env=env,
        )
        out = np.load(out_path)
        return out["cov"], out["ave"], out["amt"]


if __name__ == "__main__":
    if len(sys.argv) > 1 and sys.argv[1] == "--_worker":
        _worker_main(sys.argv[2:])
    else:
        sys.exit("usage: kernel.py --_worker IN OUT [--trace DIR]")
